# revision 5
# baseline (speedup 1.0000x reference)
"""Trainium2 Bass kernel for nn_FAM (dynamic grouped 3x3 low-pass filter + frequency gating).

Data-parallel over batch: 16 images -> 8 cores x 2 images.

v4: host-reparameterized bf16 [n,h,c,w] layout.
  - Host uploads x2 = x * s2[c] pre-transposed to [n, h, c, w] in bf16: load
    DMAs are contiguous 4KB/partition runs (HWDGE), half the bytes of f32.
  - seg tile [128h, 16c, 128w] IS s2*x: the identity matmul adds the s2*x
    term with no extra scale op. xs1 = (s1/s2)[c] * seg via 4 quarter
    tensor_tensor ops against a replicated scale const (2x-mode eligible),
    written into a [128,16,132] padded tile (data at cols 2..129).
  - rowsum via bf16 fold tree (2 TT adds) + tensor_reduce; pooled-sum carries
    s2 which is folded out of wtd/mbrow on host.
  - conv per seg: per qp one [128,1024] PSUM accumulates
      sum_dx G_(g,dx)^T @ xs1(dx-shift)   (bf16)  -> + s1*low
    + I^T @ seg                           (bf16)  -> + s2*x
    route A: + ones^T @ beta-row (f32r K=1)       -> + beta; evac = ACT copy
    route B: evac = DVE tensor_tensor add of beta broadcast
  - stores: bf16 [n,h,c,w] HWDGE on the ACT ring; host transposes back to
    [n,c,h,w] f32.

Math: s1 = (ia+1)(ll+1)-(lh+1), s2 = lh+1, beta = -ia*(ll+1)*mean(x[c]).
out = s1*low + s2*x + beta.
"""

import os
import sys

for _p in ("/opt/trn_rl_repo", "/opt/pypackages"):
    if _p not in sys.path and os.path.isdir(_p):
        sys.path.append(_p)

from contextlib import ExitStack

import numpy as np
import ml_dtypes

import concourse.bass as bass
import concourse.tile as tile
from concourse import bacc, mybir
from concourse.bass_utils import run_bass_kernel_spmd

F32 = mybir.dt.float32
F32R = mybir.dt.float32r
BF16 = mybir.dt.bfloat16
AF = mybir.ActivationFunctionType
ALU = mybir.AluOpType
BF16_NP = ml_dtypes.bfloat16

N_CORES = 8
N_PER_CORE = 2        # images per core
C = 256               # channels
G = 8                 # groups
CG = C // G           # 32 channels per group
H = W = 128
HW = H * W
K = 3
BN_EPS = 1e-5
HG_CH = 16            # channels per segment
N_HG = C // HG_CH     # 16 segments per image
WP = W + 4            # 132: xs1 padded width; data at cols 2..129

# segments whose bias rides the PE (f32r matmul) and evac is an ACT copy;
# the rest get the bias via the DVE evac add.
ROUTE_A = [
    {0, 3, 6, 9, 12, 15},   # image 0 (DVE also busy with img1 rowsums)
    {0, 5, 10, 15},         # image 1
]


def _reflect(i: int) -> int:
    if i < 0:
        return -i
    if i > H - 1:
        return 2 * (H - 1) - i
    return i


def _host_consts(conv_w, bn_gamma, bn_beta, bn_mean, bn_var, lamb_l, lamb_h,
                 inside_all):
    """Host-side parameter prep (no x-dependent math)."""
    s_bn = bn_gamma / np.sqrt(bn_var + BN_EPS)
    bn_scale = (s_bn / HW).astype(np.float32)
    bn_bias = (bn_beta - bn_mean * s_bn).astype(np.float32)
    bnsb = np.stack([bn_scale, bn_bias], axis=1)          # [72, 2]

    s1 = ((inside_all + 1.0) * (lamb_l + 1.0) - (lamb_h + 1.0)).astype(np.float64)
    s2 = (lamb_h + 1.0).astype(np.float64)
    r12 = (s1 / s2).astype(np.float32)                    # [256]
    mb = -inside_all * (lamb_l + 1.0) / HW
    # beta-row multiplier vs device prow (= s2 * sum(x)): beta = mb*sum = prow*mb/s2
    mbrow = (mb / s2).astype(np.float32).reshape(1, 256).copy()

    d_up = np.zeros((128, 128), np.float32)
    d_dn = np.zeros((128, 128), np.float32)
    idn = np.eye(128, dtype=np.float32)
    for h in range(H):
        d_up[_reflect(h - 1), h] = 1.0
        d_dn[_reflect(h + 1), h] = 1.0
    dmats = np.concatenate([d_up, idn, d_dn], axis=1)     # [128, 384]

    # filt conv weights, folded by 1/s2 (prow carries s2)
    wt = (conv_w.T.astype(np.float64) / s2[:, None]).astype(np.float32)  # [256, 72]
    wtd = np.concatenate([wt[:128], wt[128:]], axis=1)    # [128, 144]

    # replicated xs1 scale: [128 partitions, 256 ch, 32 w] bf16
    r12q = np.ascontiguousarray(
        np.broadcast_to(r12.astype(BF16_NP)[None, :, None], (128, 256, 32)))

    idn_bf = np.eye(128, dtype=np.float32).astype(BF16_NP)

    return dict(dmats=dmats, mbrow=mbrow, wtd=wtd, bnsb=bnsb, r12q=r12q,
                idn_bf=idn_bf), s2.astype(np.float32)


def _build_kernel(ctx: ExitStack, tc: "tile.TileContext",
                  x_ap: bass.AP, out_ap: bass.AP,
                  dmats_ap: bass.AP, mbrow_ap: bass.AP, wtd_ap: bass.AP,
                  bnsb_ap: bass.AP, r12q_ap: bass.AP, idnbf_ap: bass.AP):
    nc = tc.nc

    cpool = ctx.enter_context(tc.tile_pool(name="consts", bufs=1))
    stpool = ctx.enter_context(tc.tile_pool(name="stats", bufs=1))
    segpool = ctx.enter_context(tc.tile_pool(name="seg", bufs=28))
    xspool = ctx.enter_context(tc.tile_pool(name="xs1", bufs=4))
    opool = ctx.enter_context(tc.tile_pool(name="outst", bufs=3))
    rfpool = ctx.enter_context(tc.tile_pool(name="rf", bufs=2))
    rf2pool = ctx.enter_context(tc.tile_pool(name="rf2", bufs=2))
    gtmpool = ctx.enter_context(tc.tile_pool(name="gtmp", bufs=2))
    mpsum = ctx.enter_context(tc.tile_pool(name="mpsum", bufs=3, space="PSUM"))
    spsum = ctx.enter_context(tc.tile_pool(name="spsum", bufs=2, space="PSUM"))

    # ---- constants to SBUF ----
    dmats_sb = cpool.tile([128, 384], F32)
    nc.sync.dma_start(dmats_sb[:], dmats_ap)
    mbrow_sb = cpool.tile([1, 256], F32)
    nc.sync.dma_start(mbrow_sb[:], mbrow_ap)
    wtd_sb = cpool.tile([128, 144], F32)
    nc.sync.dma_start(wtd_sb[:], wtd_ap)
    bnsb_sb = cpool.tile([72, 2], F32)
    nc.sync.dma_start(bnsb_sb[:], bnsb_ap)
    r12q_sb = cpool.tile([128, 256, 32], BF16)
    nc.sync.dma_start(r12q_sb[:], r12q_ap)
    idn_bf = cpool.tile([128, 128], BF16)
    nc.sync.dma_start(idn_bf[:], idnbf_ap)
    ones_sb = cpool.tile([1, 128], F32)
    nc.vector.memset(ones_sb[:], 1.0)
    onescol = cpool.tile([128, 1], F32)
    nc.vector.memset(onescol[:], 1.0)
    onesrow_r = cpool.tile([1, 128], F32R)
    nc.vector.tensor_copy(onesrow_r[:], ones_sb[:])

    idn = dmats_sb[:, 128:256]                            # [128,128] identity f32

    # persistent per-image tiles
    rsum, fbs, b_n, b_nr, gt, prow, brow = {}, {}, {}, {}, {}, {}, {}
    for n in range(N_PER_CORE):
        rsum[n] = stpool.tile([128, 256], F32, name=f"rsum_{n}")
        fbs[n] = stpool.tile([128, 72], F32, name=f"fbs_{n}")
        b_n[n] = stpool.tile([128, 256], F32, name=f"bn_{n}")
        b_nr[n] = stpool.tile([1, 256], F32R, name=f"bnr_{n}")
        gt[n] = stpool.tile([128, G * 3 * 128], BF16, name=f"gt_{n}")
        prow[n] = stpool.tile([1, 256], F32, name=f"prow_{n}")
        brow[n] = stpool.tile([1, 256], F32, name=f"brow_{n}")

    segs = {}   # (n, hg) -> seg tile

    def load_seg(n, hg):
        c0 = hg * HG_CH
        seg = segpool.tile([128, HG_CH, W], BF16, name="seg", tag="seg")
        segs[(n, hg)] = seg
        nc.sync.dma_start(seg[:], x_ap[n, :, c0:c0 + HG_CH, :])

    def rowsum(n, hg):
        """pooled row-sums via bf16 fold tree + reduce (all DVE)."""
        c0 = hg * HG_CH
        seg = segs[(n, hg)]
        rf = rfpool.tile([128, HG_CH, 64], BF16, name="rf", tag="rf")
        nc.vector.tensor_tensor(out=rf[:], in0=seg[:, :, 0:64],
                                in1=seg[:, :, 64:128], op=ALU.add)
        rf2 = rf2pool.tile([128, HG_CH, 32], BF16, name="rf2", tag="rf2")
        nc.vector.tensor_tensor(out=rf2[:], in0=rf[:, :, 0:32],
                                in1=rf[:, :, 32:64], op=ALU.add)
        nc.vector.tensor_reduce(
            out=rsum[n][:, c0:c0 + HG_CH], in_=rf2[:],
            axis=mybir.AxisListType.X, op=ALU.add)

    def filt_branch(n):
        # pooled_row[1, c] = sum_h rsum[h, c]   (= s2[c] * sum_hw x)
        prp = spsum.tile([1, 256], F32, name="prp", tag="sp")
        nc.tensor.matmul(prp[:], lhsT=onescol[:], rhs=rsum[n][:],
                         start=True, stop=True)
        nc.scalar.copy(prow[n][:], prp[:])

        # conv: fpre[j] = sum_c wtd[c, j] * prow[c]
        fpre = spsum.tile([72, 1], F32, name="fpre", tag="sp")
        for b in range(2):
            pcp = spsum.tile([128, 1], F32, name="pcp", tag="sp")
            nc.tensor.transpose(pcp[:], prow[n][0:1, b * 128:(b + 1) * 128],
                                idn[0:1, 0:1])
            pcol = stpool.tile([128, 1], F32, name=f"pcol_{n}_{b}")
            nc.scalar.copy(pcol[:], pcp[:])
            nc.tensor.matmul(fpre[:], lhsT=wtd_sb[:, b * 72:(b + 1) * 72],
                             rhs=pcol[:], start=(b == 0), stop=(b == 1))
        filt_sb = stpool.tile([72, 1], F32, name=f"filt_{n}")
        nc.scalar.activation(filt_sb[:], fpre[:], AF.Tanh,
                             bias=bnsb_sb[:, 1:2], scale=bnsb_sb[:, 0:1])
        # transpose [72,1] -> [1,72], then broadcast to [128,72]
        ftp = spsum.tile([1, 72], F32, name="ftp", tag="sp")
        nc.tensor.transpose(ftp[:], filt_sb[:], idn[0:72, 0:72])
        filt_row = stpool.tile([1, 72], F32, name=f"filtrow_{n}")
        nc.scalar.copy(filt_row[:], ftp[:])
        fbp = spsum.tile([128, 72], F32, name="fbp", tag="sp")
        nc.tensor.matmul(fbp[:], lhsT=ones_sb[:], rhs=filt_row[:],
                         start=True, stop=True)
        nc.scalar.copy(fbs[n][:], fbp[:])

        # beta row -> broadcast b_n [128, 256] f32 + f32r row for PE bias
        nc.vector.tensor_tensor(brow[n][:], prow[n][:], mbrow_sb[:],
                                op=ALU.mult)
        nc.vector.tensor_copy(b_nr[n][:], brow[n][:])
        for b in range(2):
            bbp = spsum.tile([128, 128], F32, name="bbp", tag="sp")
            nc.tensor.matmul(bbp[:], lhsT=ones_sb[:],
                             rhs=brow[n][0:1, b * 128:(b + 1) * 128],
                             start=True, stop=True)
            nc.scalar.copy(b_n[n][:, b * 128:(b + 1) * 128], bbp[:])

    def gbuild(n, g_lo, g_hi):
        # gt[:, (g,dx), :] = sum_dy fbs[g*9+dy*3+dx] * D_dy, chunked per group
        gt4 = gt[n].rearrange("p (g dx w) -> p g dx w", g=G, dx=3)
        fb4 = fbs[n].rearrange("p (g dy dx) -> p g dy dx", g=G, dy=3)
        for g in range(g_lo, g_hi):
            tm = gtmpool.tile([128, 3, 128], BF16, name="gtmp", tag="gtmp")
            for dy in range(3):
                dmb = dmats_sb[:, dy * 128:(dy + 1) * 128][:, None, :] \
                    .broadcast_to([128, 3, 128])
                fsb = fb4[:, g, dy, :][:, :, None].broadcast_to([128, 3, 128])
                dst = gt4[:, g] if dy == 0 else tm[:]
                nc.vector.tensor_tensor(out=dst, in0=dmb, in1=fsb, op=ALU.mult)
                if dy > 0:
                    nc.vector.tensor_tensor(out=gt4[:, g], in0=gt4[:, g],
                                            in1=tm[:], op=ALU.add)

    def conv_seg(n, hg):
        c0 = hg * HG_CH
        g = c0 // CG
        seg = segs.pop((n, hg))
        route_a = hg in ROUTE_A[n]

        # JIT xs1 = (s1/s2)[c] * seg into padded tile, 4 w-quarters
        xs1 = xspool.tile([128, HG_CH, WP], BF16, name="xs1", tag="xs1")
        for q4 in range(4):
            eng = nc.vector if q4 < 2 else nc.gpsimd
            eng.tensor_tensor(
                out=xs1[:, :, 2 + 32 * q4:2 + 32 * (q4 + 1)],
                in0=seg[:, :, 32 * q4:32 * (q4 + 1)],
                in1=r12q_sb[:, c0:c0 + HG_CH, :], op=ALU.mult)
        # reflect pad cols: col1 = w=1 (data col 3), col130 = w=126 (col 128)
        nc.gpsimd.tensor_copy(xs1[:, :, 1:2], xs1[:, :, 3:4])
        nc.gpsimd.tensor_copy(xs1[:, :, WP - 2:WP - 1], xs1[:, :, WP - 4:WP - 3])

        outst = opool.tile([128, HG_CH, W], BF16, name="outst")
        ps = {}
        for qp in range(2):
            ps[qp] = mpsum.tile([128, 8 * W], F32, name="ps", tag="ps")
        # dx-major across both qp halves to minimize weight reloads;
        # matmul output limited to 512 f32 (one PSUM bank) -> qi halves
        for dx in range(3):
            blk = gt[n][:, (g * 3 + dx) * 128:(g * 3 + dx + 1) * 128]
            for qp in range(2):
                for qi in range(2):
                    c4 = 8 * qp + 4 * qi
                    nc.tensor.matmul(
                        ps[qp][:, qi * 512:(qi + 1) * 512], lhsT=blk,
                        rhs=xs1[:, c4:c4 + 4, dx + 1:dx + 129],
                        start=(dx == 0), stop=False)
        for qp in range(2):
            for qi in range(2):
                c4 = 8 * qp + 4 * qi
                nc.tensor.matmul(
                    ps[qp][:, qi * 512:(qi + 1) * 512], lhsT=idn_bf[:],
                    rhs=seg[:, c4:c4 + 4, :],
                    start=False, stop=not route_a)
        if route_a:
            for qp in range(2):
                for bi in range(2):
                    nc.tensor.matmul(
                        ps[qp][:, bi * 512:(bi + 1) * 512], lhsT=onesrow_r[:],
                        rhs=b_nr[n][0:1, c0 + 8 * qp + 4 * bi:
                                    c0 + 8 * qp + 4 * bi + 4][:, :, None]
                            .broadcast_to([1, 4, 128]),
                        start=False, stop=True)
        for qp in range(2):
            ps3 = ps[qp].rearrange("p (c w) -> p c w", c=8)
            if route_a:
                nc.scalar.copy(outst[:, 8 * qp:8 * qp + 8, :], ps3[:])
            else:
                nc.vector.tensor_tensor(
                    out=outst[:, 8 * qp:8 * qp + 8, :], in0=ps3[:],
                    in1=b_n[n][:, c0 + 8 * qp:c0 + 8 * qp + 8][:, :, None]
                        .broadcast_to([128, 8, W]),
                    op=ALU.add)
        nc.scalar.dma_start(out_ap[n, :, c0:c0 + HG_CH, :], outst[:])

    # ---------- schedule ----------
    for hg in range(N_HG):
        load_seg(0, hg)
    for hg in range(12):
        load_seg(1, hg)       # last 4 issued inside the conv-0 loop
    for hg in range(N_HG):
        rowsum(0, hg)
    filt_branch(0)
    gbuild(0, 0, 8)
    for hg in range(N_HG):
        if hg < 4:
            load_seg(1, 12 + hg)
        conv_seg(0, hg)
        if hg < 14:
            rowsum(1, hg)
        elif hg == 14:
            rowsum(1, 14)
            rowsum(1, 15)
        else:
            filt_branch(1)
            gbuild(1, 0, 2)
    for hg in range(N_HG):
        if hg < 6:
            gbuild(1, 2 + hg, 3 + hg)
        conv_seg(1, hg)


def build_nc():
    nc = bacc.Bacc("TRN2", target_bir_lowering=False, debug=False)
    x_h = nc.dram_tensor("xt", [N_PER_CORE, H, C, W], BF16, kind="ExternalInput")
    dmats_h = nc.dram_tensor("dmats", [128, 384], F32, kind="ExternalInput")
    mbrow_h = nc.dram_tensor("mbrow", [1, 256], F32, kind="ExternalInput")
    wtd_h = nc.dram_tensor("wtd", [128, 144], F32, kind="ExternalInput")
    bnsb_h = nc.dram_tensor("bnsb", [72, 2], F32, kind="ExternalInput")
    r12q_h = nc.dram_tensor("r12q", [128, 256, 32], BF16, kind="ExternalInput")
    idnbf_h = nc.dram_tensor("idn_bf", [128, 128], BF16, kind="ExternalInput")
    out_h = nc.dram_tensor("out", [N_PER_CORE, H, C, W], BF16,
                           kind="ExternalOutput")

    with tile.TileContext(nc) as tc:
        with ExitStack() as ctx:
            _build_kernel(ctx, tc, x_h.ap(), out_h.ap(), dmats_h.ap(),
                          mbrow_h.ap(), wtd_h.ap(), bnsb_h.ap(), r12q_h.ap(),
                          idnbf_h.ap())
    nc.compile()
    return nc


def _ensure_ntff_hook():
    """This image's antenv lacks axon_hooks; bass_utils imports it when
    trace=True. Inject a minimal module + register the ctypes NTFF hook the
    same way trn_boot would. Only used for local profiling runs."""
    try:
        import antenv.axon_hooks  # noqa: F401
        return
    except ImportError:
        pass
    import types
    try:
        import antenv
    except ImportError:
        return
    mod = types.ModuleType("antenv.axon_hooks")
    _state = {"hook": None}
    mod.set_axon_ntff_profile_hook = lambda h: _state.__setitem__("hook", h)
    mod.get_axon_ntff_profile_hook = lambda: _state["hook"]
    sys.modules["antenv.axon_hooks"] = mod
    antenv.axon_hooks = mod
    try:
        from trn_agent_boot.trn_boot import _ntff_profile_via_ctypes
        hook = _ntff_profile_via_ctypes("/opt/axon/libaxon_pjrt.so")
        if hook is not None:
            mod.set_axon_ntff_profile_hook(hook)
    except Exception:
        pass


def kernel(x, conv_w, bn_gamma, bn_beta, bn_mean, bn_var, lamb_l, lamb_h,
           inside_all, _trace=False, _trace_kwargs=None):
    x = np.asarray(x, dtype=np.float32)
    consts, s2 = _host_consts(conv_w, bn_gamma, bn_beta, bn_mean, bn_var,
                              lamb_l, lamb_h, inside_all)
    # reparameterized input: x2[n,h,c,w] = s2[c] * x, bf16
    x2t = (x * s2[None, :, None, None]).transpose(0, 2, 1, 3).astype(BF16_NP)
    nc = build_nc()
    in_maps = []
    for i in range(N_CORES):
        m = {"xt": np.ascontiguousarray(x2t[i * N_PER_CORE:(i + 1) * N_PER_CORE])}
        m.update(consts)
        in_maps.append(m)
    kw = {}
    if _trace:
        _ensure_ntff_hook()
        kw["trace"] = True
        if _trace_kwargs:
            kw.update(_trace_kwargs)
    res = run_bass_kernel_spmd(nc, in_maps, list(range(N_CORES)), **kw)
    out_t = np.concatenate([res.results[i]["out"] for i in range(N_CORES)],
                           axis=0)                     # [16, 128, 256, 128] bf16
    out = out_t.astype(np.float32).transpose(0, 2, 1, 3)
    out = np.ascontiguousarray(out)
    if _trace:
        kernel.last_results = res
    return out
